# revision 27
# baseline (speedup 1.0000x reference)
"""Multi-head attention (b=16, l=1025, d=768, H=12) on 8 TRN2 NeuronCores.

Sharding: data-parallel over batch - 2 batch elements per core, no
collectives.

Per-core kernel (per batch element), layouts transposed so the sequence
dim is the matmul free dim:
  1. QK^T = (Wqk stationary) @ X^T            -> [1536, L]  (bf16)
  2. V    = (X^T blocks stationary) @ Wv      -> [L, 768] stored per-head
     as [L, 12*(64+1)] with a ones column per head (gives softmax sums).
  3. Per head pair g (heads 2g, 2g+1 at partitions 0-63 / 64-127, PE
     row-group packed), per i-chunk c in {[0:512], [512:1024]}:
       S^T[jblk, i] = K_h^T.T @ Q_h^T   (K=64)
       P^T = exp(S^T/8) on ACT, grouped 2 j-tiles per instruction
       O_aug^T[65, i] += V_aug[jblk].T @ P^T[jblk]  (row 64 = denom)
       O^T = O_aug^T[0:64] * recip(O_aug^T[64])
     query col 1024 handled via a [128, 18] psum tile + one exp.
  4. Y^T = (Wo stationary) @ O^T + bo         -> [768, L] fp32 -> DRAM

Projection work of element 1 is interleaved into element 0's attention
pairs (and elem0's output projection into elem1's attention) so the PE
has dense work while ACT drains the exps.
"""

import contextlib

import numpy as np
import ml_dtypes

import concourse.bass as bass
import concourse.bacc as bacc
import concourse.mybir as mybir
import concourse.tile as tile
from concourse.bass_utils import run_bass_kernel_spmd

N_CORES = 8
B = 16
L = 1025
D = 768
H = 12
DH = 64
BPC = B // N_CORES
KT = D // 128   # 6 contraction tiles
JT = (L + 127) // 128  # 9 j-tiles; last has 1 row
SCALE = 1.0 / np.sqrt(DH)

BF16 = mybir.dt.bfloat16
F32 = mybir.dt.float32
EXP = mybir.ActivationFunctionType.Exp
MULT = mybir.AluOpType.mult
ADD = mybir.AluOpType.add

_CACHE = {}


def _build():
    nc = bacc.Bacc("TRN2", target_bir_lowering=False, debug=False,
                   num_devices=N_CORES)
    xT = nc.dram_tensor("xT", [BPC, D, L], BF16, kind="ExternalInput")
    w_qk = nc.dram_tensor("w_qk", [D, 2 * D], BF16, kind="ExternalInput")
    w_v = nc.dram_tensor("w_v", [D, D], BF16, kind="ExternalInput")
    w_o = nc.dram_tensor("w_o", [D, D], BF16, kind="ExternalInput")
    b_qk = nc.dram_tensor("b_qk", [2 * D, 1], F32, kind="ExternalInput")
    b_v = nc.dram_tensor("b_v", [1, D], F32, kind="ExternalInput")
    b_o = nc.dram_tensor("b_o", [D, 1], F32, kind="ExternalInput")
    yT = nc.dram_tensor("yT", [BPC, D, L], F32, kind="ExternalOutput")
    kTo = nc.dram_tensor("kTo", [BPC, D, L], BF16, kind="ExternalOutput")
    vo = nc.dram_tensor("vo", [BPC, JT, 128, H * (DH + 1)], BF16,
                        kind="ExternalOutput")

    with tile.TileContext(nc) as tc:
        _emit(nc, tc, xT, w_qk, w_v, w_o, b_qk, b_v, b_o, yT, kTo, vo)
    nc.compile()
    return nc


def _ap(t, poff, pcount, foff, fdims):
    """AP on tile t at partition offset poff (count pcount), free offset
    foff with free dims [(step, count), ...]."""
    base = t[:]
    pstep = base.ap[0][0]
    return bass.AP(tensor=base.tensor,
                   offset=base.offset + poff * pstep + foff,
                   ap=[[pstep, pcount]] + [list(d) for d in fdims])


def _emit(nc, tc, xT, w_qk, w_v, w_o, b_qk, b_v, b_o, yT, kTo, vo):
    ctx = contextlib.ExitStack()
    with ctx:
        consts = ctx.enter_context(tc.tile_pool(name="consts", bufs=1))
        xpool = ctx.enter_context(tc.tile_pool(name="xpool", bufs=1))
        qkpool = ctx.enter_context(tc.tile_pool(name="qkpool", bufs=2))
        vpool = ctx.enter_context(tc.tile_pool(name="vpool", bufs=2))
        otpool = ctx.enter_context(tc.tile_pool(name="otpool", bufs=2))
        ytpool = ctx.enter_context(tc.tile_pool(name="ytpool", bufs=2))
        ptpool = ctx.enter_context(tc.tile_pool(name="ptpool", bufs=6))
        smpool = ctx.enter_context(tc.tile_pool(name="smpool", bufs=3))
        # PSUM: big 2x[128,1024]=4 banks, small 1, acc 3 -> 8
        bigp = ctx.enter_context(tc.tile_pool(name="bigp", bufs=2, space="PSUM"))
        smallp = ctx.enter_context(tc.tile_pool(name="smallp", bufs=2, space="PSUM"))
        accp = ctx.enter_context(tc.tile_pool(name="accp", bufs=2, space="PSUM"))

        # ---- constants (xt emitted first by the schedule; wo last) ----
        wqk_t = [consts.tile([128, 2 * D], BF16, name=f"wqk{k}") for k in range(KT)]
        wv_t = [consts.tile([128, D], BF16, name=f"wv{k}") for k in range(KT)]
        wo_t = [consts.tile([128, D], BF16, name=f"wo{k}") for k in range(KT)]
        bqk_t = [consts.tile([128, 1], F32, name=f"bqk{m}") for m in range(2 * KT)]
        bo_t = [consts.tile([128, 1], F32, name=f"bo{m}") for m in range(KT)]
        bv_bc = consts.tile([128, D], F32, name="bvbc")

        def load_consts():
            # wv tiles race the first v_proj matmuls; interleaved k-order
            for k in range(KT):
                nc.sync.dma_start(out=wv_t[k][:], in_=w_v[k * 128:(k + 1) * 128, :])
            bva = b_v[:]
            nc.sync.dma_start(out=bv_bc[:], in_=bass.AP(
                tensor=bva.tensor, offset=bva.offset,
                ap=[[0, 128], list(bva.ap[1])]))
            for k in range(KT):
                nc.sync.dma_start(out=wqk_t[k][:], in_=w_qk[k * 128:(k + 1) * 128, :])
            for m in range(2 * KT):
                nc.sync.dma_start(out=bqk_t[m][:], in_=b_qk[m * 128:(m + 1) * 128, :])
            for m in range(KT):
                nc.sync.dma_start(out=bo_t[m][:], in_=b_o[m * 128:(m + 1) * 128, :])
            for k in range(KT):
                nc.sync.dma_start(out=wo_t[k][:], in_=w_o[k * 128:(k + 1) * 128, :])

        xt = {}
        qkT = {}
        vt = {}
        oT = {}

        def load_x(e):
            xt[e] = [xpool.tile([128, L], BF16, tag=f"xt{k}", name=f"xt{e}_{k}")
                     for k in range(KT)]
            for k in range(KT):
                nc.sync.dma_start(out=xt[e][k][:],
                                  in_=xT[e, k * 128:(k + 1) * 128, :])

        def v_proj(e, jlist):
            """V[j,:] for j-tiles in jlist; layout [jlen, 12*(64+1)]."""
            if e not in vt:
                vt[e] = [vpool.tile([128, H * (DH + 1)], BF16, tag=f"vt{j}",
                                    name=f"vt{e}_{j}") for j in range(JT)]
            for j in jlist:
                jlen = min(128, L - j * 128)
                nc.vector.memset(
                    _ap(vt[e][j], 0, 128, DH, [[DH + 1, H], [1, 1]]), 1.0)
                ps = bigp.tile([128, 1024], F32, tag="big", name=f"vps{e}_{j}")
                for k in range(KT):
                    nc.tensor.matmul(ps[:jlen, 0:512],
                                     xt[e][k][:, j * 128:j * 128 + jlen],
                                     wv_t[k][:, 0:512],
                                     start=(k == 0), stop=(k == KT - 1))
                for k in range(KT):
                    nc.tensor.matmul(ps[:jlen, 512:768],
                                     xt[e][k][:, j * 128:j * 128 + jlen],
                                     wv_t[k][:, 512:768],
                                     start=(k == 0), stop=(k == KT - 1))
                dst = _ap(vt[e][j], 0, jlen, 0, [[DH + 1, H], [1, DH]])
                src = _ap(ps, 0, jlen, 0, [[DH, H], [1, DH]])
                bia = _ap(bv_bc, 0, jlen, 0, [[DH, H], [1, DH]])
                nc.vector.tensor_tensor(out=dst, in0=src, in1=bia, op=ADD)
                nc.sync.dma_start(out=vo[e, j], in_=vt[e][j][:])

        def qk_unit(e, m):
            """One QK^T m-tile: big psum (c0+c1), small straggler col."""
            if e not in qkT:
                qkT[e] = [qkpool.tile([128, L], BF16, tag=f"qkT{t}",
                                      name=f"qkT{e}_{t}") for t in range(2 * KT)]
            ps = bigp.tile([128, 1024], F32, tag="big", name=f"qkps{e}_{m}")
            for k in range(KT):
                nc.tensor.matmul(ps[:, 0:512],
                                 wqk_t[k][:, m * 128:(m + 1) * 128],
                                 xt[e][k][:, 0:512],
                                 start=(k == 0), stop=(k == KT - 1))
            for k in range(KT):
                nc.tensor.matmul(ps[:, 512:1024],
                                 wqk_t[k][:, m * 128:(m + 1) * 128],
                                 xt[e][k][:, 512:1024],
                                 start=(k == 0), stop=(k == KT - 1))
            nc.vector.tensor_scalar_add(qkT[e][m][:, 0:1024], ps[:, 0:1024],
                                        bqk_t[m][:])
            if m >= KT:
                sg = smallp.tile([128, 512], F32, tag="small",
                                 name=f"qksg{e}_{m}")
                for k in range(KT):
                    nc.tensor.matmul(sg[:, 0:1],
                                     wqk_t[k][:, m * 128:(m + 1) * 128],
                                     xt[e][k][:, 1024:1025],
                                     start=(k == 0), stop=(k == KT - 1))
                nc.vector.tensor_scalar_add(qkT[e][m][:, 1024:1025],
                                            sg[:, 0:1], bqk_t[m][:])
                nc.sync.dma_start(out=kTo[e, (m - KT) * 128:(m - KT + 1) * 128, :],
                                  in_=qkT[e][m][:])

        def small_chunk(name, nmm, mms, dve):
            """One projection chunk through the 1-bank small psum pool."""
            ps = smallp.tile([128, 512], F32, tag="small", name=name)
            for i in range(nmm):
                mms(ps, i)
                if i % 2 == 1:
                    yield
            dve(ps)

        def v_unit_gen(e, j):
            if e not in vt:
                vt[e] = [vpool.tile([128, H * (DH + 1)], BF16, tag=f"vt{t}",
                                    name=f"vt{e}_{t}") for t in range(JT)]
            jlen = min(128, L - j * 128)
            nc.vector.memset(
                _ap(vt[e][j], 0, 128, DH, [[DH + 1, H], [1, 1]]), 1.0)
            for c, (c0, nh) in enumerate(((0, 8), (512, 4))):
                def mms(ps, k, c0=c0, clen=64 * nh):
                    nc.tensor.matmul(ps[:jlen, 0:clen],
                                     xt[e][k][:, j * 128:j * 128 + jlen],
                                     wv_t[k][:, c0:c0 + clen],
                                     start=(k == 0), stop=(k == KT - 1))
                def dve(ps, c0=c0, nh=nh):
                    dst = _ap(vt[e][j], 0, jlen, (c0 // 64) * (DH + 1),
                              [[DH + 1, nh], [1, DH]])
                    src = _ap(ps, 0, jlen, 0, [[DH, nh], [1, DH]])
                    bia = _ap(bv_bc, 0, jlen, c0, [[DH, nh], [1, DH]])
                    nc.vector.tensor_tensor(out=dst, in0=src, in1=bia, op=ADD)
                yield from small_chunk(f"vg{e}_{j}_{c}", KT, mms, dve)
            nc.sync.dma_start(out=vo[e, j], in_=vt[e][j][:])

        def qk_unit_gen(e, m):
            if e not in qkT:
                qkT[e] = [qkpool.tile([128, L], BF16, tag=f"qkT{t}",
                                      name=f"qkT{e}_{t}") for t in range(2 * KT)]
            for c in range(2):
                def mms(ps, k, c=c):
                    nc.tensor.matmul(ps[:, 0:512],
                                     wqk_t[k][:, m * 128:(m + 1) * 128],
                                     xt[e][k][:, c * 512:c * 512 + 512],
                                     start=(k == 0), stop=(k == KT - 1))
                def dve(ps, c=c):
                    nc.vector.tensor_scalar_add(
                        qkT[e][m][:, c * 512:c * 512 + 512],
                        ps[:, 0:512], bqk_t[m][:])
                yield from small_chunk(f"qg{e}_{m}_{c}", KT, mms, dve)
            if m >= KT:
                def mms(ps, k):
                    nc.tensor.matmul(ps[:, 0:1],
                                     wqk_t[k][:, m * 128:(m + 1) * 128],
                                     xt[e][k][:, 1024:1025],
                                     start=(k == 0), stop=(k == KT - 1))
                def dve(ps):
                    nc.vector.tensor_scalar_add(qkT[e][m][:, 1024:1025],
                                                ps[:, 0:1], bqk_t[m][:])
                yield from small_chunk(f"qgs{e}_{m}", KT, mms, dve)
                nc.sync.dma_start(
                    out=kTo[e, (m - KT) * 128:(m - KT + 1) * 128, :],
                    in_=qkT[e][m][:])

        def out_unit_gen(e, m):
            yt = ytpool.tile([128, L], F32, tag="yt", name=f"yt{e}_{m}")
            for c in range(2):
                def mms(ps, k, c=c):
                    nc.tensor.matmul(ps[:, 0:512],
                                     wo_t[k][:, m * 128:(m + 1) * 128],
                                     oT[e][k][:, c * 512:c * 512 + 512],
                                     start=(k == 0), stop=(k == KT - 1))
                def dve(ps, c=c):
                    nc.vector.tensor_scalar_add(yt[:, c * 512:c * 512 + 512],
                                                ps[:, 0:512], bo_t[m][:])
                yield from small_chunk(f"og{e}_{m}_{c}", KT, mms, dve)
                nc.sync.dma_start(
                    out=yT[e, m * 128:(m + 1) * 128, c * 512:c * 512 + 512],
                    in_=yt[:, c * 512:c * 512 + 512])

        def load_x_gen(e):
            load_x(e)
            yield

        class Fill:
            def __init__(self, gens):
                self.gens = list(gens)

            def pull(self, n=1):
                while n > 0 and self.gens:
                    try:
                        next(self.gens[0])
                        n -= 1
                    except StopIteration:
                        self.gens.pop(0)

            def finish(self, k):
                """Exhaust the first k remaining generators."""
                for gen in self.gens[:k]:
                    for _ in gen:
                        pass
                self.gens = self.gens[k:]

            def flush(self):
                self.finish(len(self.gens))

        def _fill(filler, n=1):
            for _ in range(n):
                if filler:
                    filler.pop(0)()

        def attention(e, g, fill=None):
            """Head pair g: heads 2g (partitions 0-63), 2g+1 (64-127)."""
            fill = fill or Fill([])
            if e not in oT:
                oT[e] = [otpool.tile([128, L], BF16, tag=f"oT{t}",
                                     name=f"oT{e}_{t}") for t in range(KT)]
            kt_q, kt_k = qkT[e][g], qkT[e][KT + g]
            for (i0, ilen) in ((0, 512), (512, 512)):
                oacc = [accp.tile([128, 512], F32, tag="acc",
                                  name=f"oacc{e}_{g}_{i0}_{u}") for u in range(2)]
                # Per key-block j one big tile holds u0 scores (cols
                # 0:512) and u1 scores (cols 512:1024); the two K=64 mms are
                # adjacent instructions in disjoint PE row groups (0-63 /
                # 64-127) so they overlap on hardware. PV runs 2 key-blocks
                # behind scores (2 big slots); filler plugs residual stalls.
                pts = []

                def pv(j):
                    pt = pts[j]
                    for u in range(2):
                        h = 2 * g + u
                        nc.tensor.matmul(
                            oacc[u][:DH + 1, :ilen],
                            vt[e][j][:, h * (DH + 1):(h + 1) * (DH + 1)],
                            pt[:, u * 512:u * 512 + ilen],
                            start=(j == 0), stop=False)

                for j in range(8):
                    if j >= 2:
                        pv(j - 2)
                    sps = bigp.tile([128, 1024], F32, tag="big",
                                    name=f"sps{e}_{g}_{i0}_{j}")
                    for u in range(2):
                        nc.tensor.matmul(
                            sps[:128, u * 512:u * 512 + ilen],
                            kt_k[u * 64:(u + 1) * 64, j * 128:(j + 1) * 128],
                            kt_q[u * 64:(u + 1) * 64, i0:i0 + ilen],
                            start=True, stop=True)
                    pt = ptpool.tile([128, 1024], BF16, tag="pt",
                                     name=f"pt{e}_{g}_{i0}_{j}")
                    nc.scalar.activation(pt[:, :], sps[:, :], EXP,
                                         bias=0.0, scale=float(SCALE))
                    pts.append(pt)
                    fill.pull(1)
                pv(6)
                fill.pull(1)
                pv(7)
                fill.pull(1)
                # j8 (jlen=1): u0 in cols 0:512, u1 in cols 512:1024,
                # both at partition 0 so PV lhsT/rhs bases match
                sp8 = bigp.tile([128, 1024], F32, tag="big",
                                name=f"sp8{e}_{g}_{i0}")
                for u in range(2):
                    nc.tensor.matmul(
                        sp8[0:1, u * 512:u * 512 + ilen],
                        kt_k[u * 64:(u + 1) * 64, 1024:1025],
                        kt_q[u * 64:(u + 1) * 64, i0:i0 + ilen],
                        start=True, stop=True)
                pt8 = ptpool.tile([1, 1024], BF16, tag="pt8",
                                  name=f"pt8{e}_{g}_{i0}")
                nc.scalar.activation(pt8[:1, :], sp8[:1, :], EXP,
                                     bias=0.0, scale=float(SCALE))
                for u in range(2):
                    h = 2 * g + u
                    nc.tensor.matmul(
                        oacc[u][:DH + 1, :ilen],
                        vt[e][JT - 1][:1, h * (DH + 1):(h + 1) * (DH + 1)],
                        pt8[0:1, u * 512:u * 512 + ilen],
                        start=False, stop=True)
                fill.pull(1)
                # normalize
                for u in range(2):
                    rec1 = smpool.tile([1, 512], F32, tag="rec1",
                                       name=f"rec1{e}_{g}_{i0}_{u}")
                    nc.vector.reciprocal(rec1[:1, :ilen],
                                         oacc[u][DH:DH + 1, :ilen])
                    rec = smpool.tile([128, 512], F32, tag="rec",
                                      name=f"rec{e}_{g}_{i0}_{u}")
                    nc.gpsimd.partition_broadcast(rec[:DH, :ilen],
                                                  rec1[:1, :ilen])
                    nc.vector.tensor_tensor(
                        out=oT[e][g][u * 64:(u + 1) * 64, i0:i0 + ilen],
                        in0=oacc[u][:DH, :ilen], in1=rec[:DH, :ilen], op=MULT)
                fill.pull(1)

        def out_proj(e, m):
            yt = ytpool.tile([128, L], F32, tag="yt", name=f"yt{e}_{m}")
            ps = bigp.tile([128, 1024], F32, tag="big", name=f"ops{e}_{m}")
            for k in range(KT):
                nc.tensor.matmul(ps[:, 0:512], wo_t[k][:, m * 128:(m + 1) * 128],
                                 oT[e][k][:, 0:512],
                                 start=(k == 0), stop=(k == KT - 1))
            for k in range(KT):
                nc.tensor.matmul(ps[:, 512:1024],
                                 wo_t[k][:, m * 128:(m + 1) * 128],
                                 oT[e][k][:, 512:1024],
                                 start=(k == 0), stop=(k == KT - 1))
            nc.vector.tensor_scalar_add(yt[:, 0:1024], ps[:, 0:1024], bo_t[m][:])
            nc.sync.dma_start(out=yT[e, m * 128:(m + 1) * 128, 0:1024],
                              in_=yt[:, 0:1024])

        # ---- schedule ----
        # warm the exp table during the input DMA shadow
        warm = smpool.tile([1, 512], F32, tag="rec1", name="warm")
        nc.vector.memset(warm[:1, 0:1], 0.0)
        nc.scalar.activation(warm[:1, 0:1], warm[:1, 0:1], EXP,
                             bias=0.0, scale=1.0)
        # interleave xt[k] / wv[k] so v_proj's k-th matmul can start as
        # soon as the k-th pair lands
        xt[0] = [xpool.tile([128, L], BF16, tag=f"xt{k}", name=f"xt0_{k}")
                 for k in range(KT)]
        for k in range(KT):
            nc.sync.dma_start(out=xt[0][k][:],
                              in_=xT[0, k * 128:(k + 1) * 128, :])
            nc.sync.dma_start(out=wv_t[k][:], in_=w_v[k * 128:(k + 1) * 128, :])
        bva = b_v[:]
        nc.sync.dma_start(out=bv_bc[:], in_=bass.AP(
            tensor=bva.tensor, offset=bva.offset,
            ap=[[0, 128], list(bva.ap[1])]))
        for k in range(KT):
            nc.sync.dma_start(out=wqk_t[k][:], in_=w_qk[k * 128:(k + 1) * 128, :])
        for m in range(2 * KT):
            nc.sync.dma_start(out=bqk_t[m][:], in_=b_qk[m * 128:(m + 1) * 128, :])
        for m in range(KT):
            nc.sync.dma_start(out=bo_t[m][:], in_=b_o[m * 128:(m + 1) * 128, :])
        for k in range(KT):
            nc.sync.dma_start(out=wo_t[k][:], in_=w_o[k * 128:(k + 1) * 128, :])
        v_proj(0, list(range(JT)))
        qk_unit(0, 0); qk_unit(0, KT)
        f = []
        for g in range(1, KT):
            f += [lambda m=g: qk_unit(0, m), lambda m=KT + g: qk_unit(0, m)]
        f += [lambda: load_x(1)]
        f += [lambda j=j: v_proj(1, [j]) for j in range(JT)]
        f += [lambda: qk_unit(1, 0), lambda: qk_unit(1, KT)]
        for g in range(KT):
            n_slots = 5
            take, f = f[:n_slots], f[n_slots:]
            attention(0, g, take)
        for fn in f:
            fn()
        f = []
        for g in range(1, KT):
            f += [lambda m=g: qk_unit(1, m), lambda m=KT + g: qk_unit(1, m)]
        f += [lambda m=m: out_proj(0, m) for m in range(KT)]
        for g in range(KT):
            n_slots = 3
            take, f = f[:n_slots], f[n_slots:]
            attention(1, g, take)
        for fn in f:
            fn()
        for m in range(KT):
            out_proj(1, m)
